# revision 37
# baseline (speedup 1.0000x reference)
"""Trainium2 Bass kernel for nn_AdaptiveAttention (8-core data parallel).

Sharding strategy (host side, inside kernel()):
  - batch dim (256) sharded over 8 cores -> 32 batches/core.
  - all weights replicated (passed bf16; compute is bf16 on the PE).
  - spatial_image shard is passed in BOTH layouts as bf16:
      xt = X^T [1024, 6272]  (features on partitions; rhs of the big matmul)
      xn = X   [6272, 1024]  (rows on partitions; rhs of the context matmul)
    Two bf16 copies == same HBM bytes as one f32 copy.
  - small per-batch tensors / biases are passed pre-transposed / pre-tiled so
    every on-chip bias is per-partition and no on-chip transposes of inputs
    are needed.

On-chip math (per core, "transposed world": features on partitions, batch on
free dim):
  saT  = relu(Wsa^T @ st^T + bsa)            [1024, 32]
  haT  = tanh(Wha^T @ dec^T + bha)           [1024, 32]
  hatT = Wht^T @ haT                          [512, 32]
  senT = tanh(Wst^T @ saT + hatT + bst+bht)   [512, 32]
  alpha_sen[b] = walpha . senT[:, b]          via PE (lhsT=senT chunk)
  V^T  = Wva^T @ X^T (+ hatT + bvt+bht bcast) -> tanh -> alpha = walpha . (.)
  softmax over [32, 197] (b on partitions) -> att_weights (output)
  context = Wblk^T @ X + e_sen * sa           (Wblk = block-diag of UNNORMALIZED
                                               exp weights: mask * exp; the
                                               1/sum normalization is folded
                                               into the context epilogue)
  out_l^T = tanh(Wctx^T @ (context^T + haT) + bctx)   [1024, 32]
"""

import numpy as np
import ml_dtypes

import concourse.bass as bass
import concourse.bacc as bacc
import concourse.tile as tile
import concourse.mybir as mybir
from concourse.bass_utils import run_bass_kernel_spmd

BF16 = mybir.dt.bfloat16
F32 = mybir.dt.float32
AF = mybir.ActivationFunctionType
ALU = mybir.AluOpType
AX = mybir.AxisListType

NCORES = 8
B, P, H, A = 256, 196, 1024, 512
NB = B // NCORES            # 32 batches per core
R = NB * P                  # 6272 rows per core
KH = H // 128               # 8 h-chunks
KA = A // 128               # 4 a-chunks
GB = 2                      # batches per V-group
NG = NB // GB               # 16 groups
GR = GB * P                 # 392 rows per group
RC = R // 128               # 49 row-chunks for context
XNC = 7                     # xn tiles consolidated per DMA

_BF = ml_dtypes.bfloat16


def build_nc():
    nc = bacc.Bacc(None, target_bir_lowering=False, debug=False)

    xt = nc.declare_dram_parameter("xt", [H, R], BF16, isOutput=False)
    xn = nc.declare_dram_parameter("xn", [R, H], BF16, isOutput=False)
    dot_t = nc.declare_dram_parameter("dot_t", [H, NB], BF16, isOutput=False)
    st_t = nc.declare_dram_parameter("st_t", [H, NB], BF16, isOutput=False)
    wsa = nc.declare_dram_parameter("wsa", [H, H], BF16, isOutput=False)
    wst = nc.declare_dram_parameter("wst", [H, A], BF16, isOutput=False)
    wha = nc.declare_dram_parameter("wha", [H, H], BF16, isOutput=False)
    wht = nc.declare_dram_parameter("wht", [H, A], BF16, isOutput=False)
    wva = nc.declare_dram_parameter("wva", [H, A], BF16, isOutput=False)
    wctx = nc.declare_dram_parameter("wctx", [H, H], BF16, isOutput=False)
    walpha = nc.declare_dram_parameter("walpha", [128, KA], BF16, isOutput=False)
    # [bsa | bha | bctx | bst+bht | bvt+bht | walpha] natural rows
    bias_nat = nc.declare_dram_parameter("bias_nat", [128, 4608], BF16, isOutput=False)

    out_o = nc.declare_dram_parameter("out_l", [NB, H], F32, isOutput=True)
    attw_o = nc.declare_dram_parameter("attw", [NB, P + 1], F32, isOutput=True)

    with tile.TileContext(nc) as tc:
        with (
            tc.tile_pool(name="const", bufs=1) as cp,
            tc.tile_pool(name="dram", bufs=1, space="DRAM") as dp,
            tc.tile_pool(name="xtp", bufs=2) as xtp,
            tc.tile_pool(name="xnp", bufs=4) as xnp,
            tc.tile_pool(name="tvp", bufs=2) as tvp,
        ):
            # ---- constant loads (sync HWDGE queue) ------------------------
            # order matters: small tensors, then weights in first-use order.
            walpha_sb = cp.tile([128, KA], BF16)
            nc.sync.dma_start(walpha_sb[:], walpha.ap())
            bias_sb = cp.tile([128, 4608], BF16)
            nc.sync.dma_start(bias_sb[:], bias_nat.ap())
            dot_sb = cp.tile([128, KH, NB], BF16)
            nc.sync.dma_start(dot_sb[:], dot_t.ap().rearrange("(k ki) b -> ki k b", ki=128))
            st_sb = cp.tile([128, KH, NB], BF16)
            nc.sync.dma_start(st_sb[:], st_t.ap().rearrange("(k ki) b -> ki k b", ki=128))

            def load_w(name, ap, cols, eng):
                t = cp.tile([128, KH, cols], BF16, name=name)
                eng.dma_start(t[:], ap.rearrange("(k ki) n -> ki k n", ki=128))
                return t

            # split weight streams across the two HWDGE queues in first-use
            # order so phase B / phase C never wait on a single queue.
            wsa_sb = load_w("wsa_sb", wsa.ap(), H, nc.sync)
            wht_sb = load_w("wht_sb", wht.ap(), A, nc.scalar)
            wst_sb = load_w("wst_sb", wst.ap(), A, nc.scalar)
            wva_sb = load_w("wva_sb", wva.ap(), A, nc.scalar)
            wha_sb = load_w("wha_sb", wha.ap(), H, nc.sync)
            wctx_sb = load_w("wctx_sb", wctx.ap(), H, nc.sync)

            ident_bf = cp.tile([128, 128], BF16)
            nc.gpsimd.memset(ident_bf[:], 0.0)
            nc.gpsimd.affine_select(
                out=ident_bf[:], in_=ident_bf[:],
                compare_op=ALU.not_equal, fill=1.0, base=0,
                pattern=[[-1, 128]], channel_multiplier=1,
            )
            ident_f = cp.tile([32, 32], F32)
            nc.gpsimd.memset(ident_f[:], 0.0)
            nc.gpsimd.affine_select(
                out=ident_f[:], in_=ident_f[:],
                compare_op=ALU.not_equal, fill=1.0, base=0,
                pattern=[[-1, 32]], channel_multiplier=1,
            )
            # block-diagonal mask: mask01[q, c, b] = 1 where row 128c+q
            # belongs to batch b (196b <= 128c+q <= 196b+195), else 0.
            mask01 = cp.tile([128, RC, NB], BF16)
            nc.gpsimd.memset(mask01[:], 1.0)
            nc.gpsimd.affine_select(
                out=mask01[:], in_=mask01[:],
                compare_op=ALU.is_ge, fill=0.0, base=0,
                pattern=[[128, RC], [-P, NB]], channel_multiplier=1)
            nc.gpsimd.affine_select(
                out=mask01[:], in_=mask01[:],
                compare_op=ALU.is_ge, fill=0.0, base=P - 1,
                pattern=[[-128, RC], [P, NB]], channel_multiplier=-1)

            # persistent small tensors
            saT = cp.tile([128, KH, NB], BF16)
            haT = cp.tile([128, KH, NB], BF16)
            hbaV = cp.tile([128, KA, NB], F32)   # h_att + b_h_att + b_v_att
            sa_scr = cp.tile([NB, H], BF16)      # scratch (ha natural)
            sen_nat = cp.tile([NB, A], BF16)
            walpha_rep = cp.tile([NB, A], BF16)
            asen = cp.tile([NB, 1], F32)
            expr_sb = cp.tile([1, R], BF16)
            expw_bp = cp.tile([NB, P + 1], F32)
            rinv_u = cp.tile([NB, 1], F32)
            exp_sen_u = cp.tile([NB, 1], F32)
            attw_sb = cp.tile([NB, P + 1], F32)
            w_r = cp.tile([128, 64], BF16)
            wblk = cp.tile([128, RC, NB], BF16)
            sa_nat = cp.tile([NB, H], BF16)
            ctx_nat = cp.tile([NB, H], F32)
            sen_term = cp.tile([NB, H], F32)
            ctxha = cp.tile([128, KH, NB], BF16)
            out_nat = cp.tile([NB, H], F32)
            w_d = dp.tile([64 * 128], BF16)
            zpad = cp.tile([1, 64 * 128 - R], BF16)
            nc.vector.memset(zpad[:], 0.0)
            nc.gpsimd.dma_start(w_d[R:], zpad[:])

            # ---- phase B: small paths (natural layout, psum-seeded bias) --
            with tc.tile_pool(name="psB", bufs=1, space="PSUM") as psB:
                def nat_path(x_sb, w_sb, bias_off, act, out_nat, outT, identTag):
                    # out_nat[32, 1024] = act(x @ W + b); also transpose to
                    # outT [128, KH, 32] if outT is not None.
                    for hf in range(2):
                        ps = psB.tile([NB, 512], F32, tag="pnat", bufs=2,
                                      name=f"psn{bias_off}_{hf}")
                        nc.tensor.matmul(
                            ps[:], ident_bf[:, :NB],
                            bias_sb[:, bias_off + hf * 512:bias_off + hf * 512 + 512],
                            start=True, stop=False)
                        for k in range(KH):
                            nc.tensor.matmul(
                                ps[:], x_sb[:, k, :],
                                w_sb[:, k, hf * 512:(hf + 1) * 512],
                                start=False, stop=(k == KH - 1))
                        nc.scalar.activation(out_nat[:, hf * 512:(hf + 1) * 512],
                                             ps[:], act)
                    if outT is not None:
                        for ko in range(KH):
                            pt = psB.tile([128, NB], BF16, tag=identTag, bufs=2,
                                          name=f"pst{bias_off}_{ko}")
                            nc.tensor.transpose(
                                pt[:], out_nat[:, ko * 128:(ko + 1) * 128],
                                ident_bf[:NB, :NB])
                            nc.vector.tensor_copy(outT[:, ko, :], pt[:])

                nat_path(st_sb, wsa_sb, 0, AF.Relu, sa_nat, saT, "ptB")
                nat_path(dot_sb, wha_sb, 1024, AF.Tanh, sa_scr, haT, "ptB")

                # walpha replicated across the 32 batch partitions
                psw = psB.tile([NB, A], F32, tag="psen", bufs=2)
                nc.tensor.matmul(psw[:], ident_bf[:, :NB], bias_sb[:, 4096:4608],
                                 start=True, stop=True)
                nc.vector.tensor_copy(walpha_rep[:], psw[:])

                # sen = tanh(sa @ Wst + ha @ Wht + bst + bht)   [32, 512]
                ps_sen = psB.tile([NB, A], F32, tag="psen", bufs=2)
                nc.tensor.matmul(ps_sen[:], ident_bf[:, :NB], bias_sb[:, 3072:3584],
                                 start=True, stop=False)
                for k in range(KH):
                    nc.tensor.matmul(ps_sen[:], haT[:, k, :], wht_sb[:, k, :],
                                     start=False, stop=False)
                for k in range(KH):
                    nc.tensor.matmul(ps_sen[:], saT[:, k, :], wst_sb[:, k, :],
                                     start=False, stop=(k == KH - 1))
                nc.scalar.activation(sen_nat[:], ps_sen[:], AF.Tanh)

                # hbaV = ha @ Wht + bvt + bht, then transpose to [a, b]
                ps_hbv = psB.tile([NB, A], F32, tag="psen", bufs=2)
                nc.tensor.matmul(ps_hbv[:], ident_bf[:, :NB], bias_sb[:, 3584:4096],
                                 start=True, stop=False)
                for k in range(KH):
                    nc.tensor.matmul(ps_hbv[:], haT[:, k, :], wht_sb[:, k, :],
                                     start=False, stop=(k == KH - 1))
                nc.scalar.copy(out_nat[:, :A], ps_hbv[:])
                for ao in range(KA):
                    ptf = psB.tile([128, NB], F32, tag="ptF", bufs=2)
                    nc.tensor.transpose(
                        ptf[:], out_nat[:, ao * 128:(ao + 1) * 128],
                        ident_f[:])
                    nc.vector.tensor_copy(hbaV[:, ao, :], ptf[:])

                # alpha_sen = sen . walpha
                nc.vector.tensor_mul(sa_scr[:, :A], sen_nat[:], walpha_rep[:])
                nc.vector.tensor_reduce(asen[:], sa_scr[:, :A], axis=AX.X,
                                        op=ALU.add)
                nc.scalar.activation(exp_sen_u[:], asen[:], AF.Exp)
                nc.vector.tensor_copy(expw_bp[:, P:P + 1], exp_sen_u[:])

            # ---- phase C: big matmul + alpha ------------------------------
            with tc.tile_pool(name="psC", bufs=1, space="PSUM") as psC:
                xt_r = xt.ap().rearrange("(k ki) r -> ki k r", ki=128)
                wd_r = w_d[:R].rearrange("(b p) -> b p", b=NB)
                for g in range(NG):
                    xt_g = xtp.tile([128, KH, GR], BF16, tag="xtg")
                    nc.scalar.dma_start(xt_g[:], xt_r[:, :, g * GR:(g + 1) * GR])
                    tvts = []
                    for ao in range(KA):
                        ps = psC.tile([128, GR], F32, tag="pv", bufs=5)
                        for k in range(KH):
                            nc.tensor.matmul(
                                ps[:], wva_sb[:, k, ao * 128:(ao + 1) * 128],
                                xt_g[:, k, :], start=(k == 0), stop=(k == KH - 1))
                        tv = tvp.tile([128, GR], BF16, tag=f"tv{ao}")
                        nc.vector.tensor_tensor(
                            tv.rearrange("p (b q) -> p b q", q=P),
                            ps.rearrange("p (b q) -> p b q", q=P),
                            hbaV[:, ao, g * GB:(g + 1) * GB, None].to_broadcast(
                                (128, GB, P)),
                            op=ALU.add)
                        tvt = tvp.tile([128, GR], BF16, tag=f"tvt{ao}")
                        nc.scalar.activation(tvt[:], tv[:], AF.Tanh)
                        tvts.append(tvt)
                    ps_a = psC.tile([1, GR], F32, tag="pa", bufs=2)
                    for ao in range(KA):
                        nc.tensor.matmul(ps_a[:], walpha_sb[:, ao:ao + 1],
                                         tvts[ao][:],
                                         start=(ao == 0), stop=(ao == KA - 1))
                    # unshifted exp(alpha) in r-major order, straight to DRAM
                    # as bf16 (gpsimd casts); feeds both the Wblk transpose
                    # and the [b, p]-layout attention-weight output.
                    nc.scalar.activation(expr_sb[:, g * GR:(g + 1) * GR],
                                         ps_a[:], AF.Exp)
                    nc.gpsimd.dma_start(w_d[g * GR:(g + 1) * GR],
                                        expr_sb[:, g * GR:(g + 1) * GR])
                    nc.gpsimd.dma_start(expw_bp[g * GB:(g + 1) * GB, :P],
                                        wd_r[g * GB:(g + 1) * GB, :])

            # ---- phase D: Wblk construction + attention-weight output -----
            with tc.tile_pool(name="psD", bufs=1, space="PSUM") as psD:
                # Wblk critical path: load the bf16 unshifted exp weights as
                # [64, 128], PE-transpose to r-major [128, 64], then one DVE
                # multiply with the precomputed block-diagonal mask.
                w_sb = cp.tile([64, 128], BF16)
                nc.scalar.dma_start(w_sb[:], w_d.rearrange("(c q) -> c q", q=128))
                pwr = psD.tile([128, 64], BF16, tag="pwr", bufs=1)
                nc.tensor.transpose(pwr[:], w_sb[:], ident_bf[:64, :64])
                nc.vector.tensor_copy(w_r[:], pwr[:])
                nc.vector.tensor_tensor(
                    wblk[:], mask01[:],
                    w_r[:, :RC, None].to_broadcast((128, RC, NB)), op=ALU.mult)

                # normalize the unshifted exp for the attention-weight output
                ssum = cp.tile([NB, 1], F32)
                nc.vector.tensor_reduce(ssum[:], expw_bp[:], axis=AX.X,
                                        op=ALU.add)
                nc.vector.reciprocal(rinv_u[:], ssum[:])
                nc.vector.tensor_scalar_mul(attw_sb[:], expw_bp[:], rinv_u[:])
                nc.gpsimd.dma_start(attw_o.ap(), attw_sb[:])

            # ---- phase E: context + final ---------------------------------
            with tc.tile_pool(name="psE", bufs=1, space="PSUM") as psE:
                pctx0 = psE.tile([NB, 512], F32, tag="pctx", bufs=2)
                pctx1 = psE.tile([NB, 512], F32, tag="pctx", bufs=2)
                xn_r = xn.ap().rearrange("(t j q) h -> t q j h", q=128, j=XNC)
                for t in range(RC // XNC):
                    xn_t = xnp.tile([128, XNC, H], BF16, tag="xnt")
                    nc.sync.dma_start(xn_t[:], xn_r[t])
                    for j in range(XNC):
                        c = t * XNC + j
                        nc.tensor.matmul(pctx0[:], wblk[:, c, :], xn_t[:, j, :512],
                                         start=(c == 0), stop=(c == RC - 1))
                        nc.tensor.matmul(pctx1[:], wblk[:, c, :], xn_t[:, j, 512:],
                                         start=(c == 0), stop=(c == RC - 1))
                # sentinel term, then fold in the unshifted-1/sum normalization
                nc.vector.tensor_scalar_mul(sen_term[:], sa_nat[:],
                                            exp_sen_u[:])
                nc.vector.tensor_tensor(ctx_nat[:, :512], pctx0[:],
                                        sen_term[:, :512], op=ALU.add)
                nc.vector.tensor_tensor(ctx_nat[:, 512:], pctx1[:],
                                        sen_term[:, 512:], op=ALU.add)
                nc.vector.tensor_scalar_mul(ctx_nat[:], ctx_nat[:], rinv_u[:])
                # transpose context to [h, b] and add haT
                for ko in range(KH):
                    pt = psE.tile([128, NB], F32, tag="ptr_cx", bufs=2)
                    nc.tensor.transpose(pt[:], ctx_nat[:, ko * 128:(ko + 1) * 128],
                                        ident_f[:])
                    nc.vector.tensor_tensor(ctxha[:, ko, :], pt[:], haT[:, ko, :],
                                            op=ALU.add)
                # out_l = tanh((ctx + ha) @ Wctx + bctx)  in natural layout
                for hf in range(2):
                    ps = psE.tile([NB, 512], F32, tag="po", bufs=2)
                    nc.tensor.matmul(ps[:], ident_bf[:, :NB],
                                     bias_sb[:, 2048 + hf * 512:2560 + hf * 512],
                                     start=True, stop=False)
                    for k in range(KH):
                        nc.tensor.matmul(
                            ps[:], ctxha[:, k, :],
                            wctx_sb[:, k, hf * 512:(hf + 1) * 512],
                            start=False, stop=(k == KH - 1))
                    nc.scalar.activation(out_nat[:, hf * 512:(hf + 1) * 512],
                                         ps[:], AF.Tanh)
                nc.gpsimd.dma_start(out_o.ap(), out_nat[:])

    nc.compile()
    return nc


_NC_CACHE = None


def _get_nc():
    global _NC_CACHE
    if _NC_CACHE is None:
        _NC_CACHE = build_nc()
    return _NC_CACHE


def _prep_in_maps(inputs):
    f32 = np.float32
    si = np.asarray(inputs["spatial_image"], f32)
    dec = np.asarray(inputs["decoder_out"], f32)
    st = np.asarray(inputs["st"], f32)

    def bf(x):
        return np.ascontiguousarray(x).astype(_BF)

    shared = {
        "wsa": bf(inputs["W_sen_aff"]),
        "wst": bf(inputs["W_sen_att"]),
        "wha": bf(inputs["W_h_aff"]),
        "wht": bf(inputs["W_h_att"]),
        "wva": bf(inputs["W_v_att"]),
        "wctx": bf(inputs["W_ctx"]),
        "walpha": bf(np.asarray(inputs["W_alpha"], f32)[:, 0].reshape(KA, 128).T),
    }
    bst_bht = (np.asarray(inputs["b_sen_att"], f32)
               + np.asarray(inputs["b_h_att"], f32))
    bvt_bht = (np.asarray(inputs["b_v_att"], f32)
               + np.asarray(inputs["b_h_att"], f32))
    bias_nat = np.concatenate([
        np.asarray(inputs["b_sen_aff"], f32),
        np.asarray(inputs["b_h_aff"], f32),
        np.asarray(inputs["b_ctx"], f32),
        bst_bht, bvt_bht,
        np.asarray(inputs["W_alpha"], f32)[:, 0],
    ])[None, :]
    shared["bias_nat"] = bf(np.repeat(bias_nat, 128, axis=0))

    in_maps = []
    for c in range(NCORES):
        sl = slice(c * NB, (c + 1) * NB)
        xs = si[sl].reshape(R, H)
        m = dict(shared)
        m["xt"] = bf(xs.T)
        m["xn"] = xs.astype(_BF)
        m["dot_t"] = bf(dec[sl].T)
        m["st_t"] = bf(st[sl].T)
        in_maps.append(m)
    return in_maps


def _run(inputs, **kwargs):
    nc = _get_nc()
    in_maps = _prep_in_maps(inputs)
    res = run_bass_kernel_spmd(nc, in_maps, core_ids=list(range(NCORES)), **kwargs)
    out_l = np.empty((B, H), np.float32)
    attw = np.empty((B, P + 1), np.float32)
    for c in range(NCORES):
        sl = slice(c * NB, (c + 1) * NB)
        out_l[sl] = np.asarray(res.results[c]["out_l"])
        attw[sl] = np.asarray(res.results[c]["attw"])
    beta = attw[:, P:P + 1].copy()
    return (out_l, attw, beta), res


def kernel(**inputs):
    outs, _ = _run(inputs)
    return outs
